# revision 30
# baseline (speedup 1.0000x reference)
"""Trainium2 Bass kernel for nn_DiscreteComm (GNN message passing).

Algorithm (matches the reference AS EXECUTED ON THIS JAX/NEURON STACK, where
jax.ops.segment_max lowers to segment-SUM with 0 identity -- verified):
  - m = y_hard + y_soft - stop_grad(y_soft) == y_hard exactly in fp arithmetic,
    so each edge contributes one BIT per message symbol:
        bit[e,j] = (zdiff[src[e],j] + g0[e,j] >= g1[e,j])
    where zdiff[v,j] = (x[v]||h[v]) . (W_enc[2j]-W_enc[2j+1]) + (b_enc[2j]-b_enc[2j+1])
  - c[v,2j] = cnt[v,j] = sum of bits over in-edges;  c[v,2j+1] = deg[v]-cnt[v,j]
  - dec = W_dec @ c + b_dec  folds to  Wdd @ cnt + w_odd (x) deg + b_dec,
    Wdd = W_dec[:,even]-W_dec[:,odd], w_odd = sum_j W_dec[:,2j+1].
  - GRU update per node (dense GEMMs, fp32 throughout; bit decisions fp32).

Distribution: edges sharded by dst range (2500 nodes/core, 8 cores). Each core
computes zdiff for its own nodes; AllGather makes the full fp32 zdiff table;
per-edge bits via indirect-DMA gather (fused +G0 via CCE add) + one DVE is_ge;
per-node counts via tiny one-hot `sel` matmuls into PSUM windows; GRU in fp32.

SPMD note: all 8 cores run one program, so the block schedule (node windows,
chunk boundaries) is shared. Per-core in-degree sequences are aligned by
relabeling each core's nodes in descending-degree order; shared slot counts
are the per-rank max over cores (~1% padding).
"""

import os
import sys

import numpy as np

sys.path.insert(0, "/opt/trn_rl_repo")

N_NODES = 20000
N_EDGES = 640000
HIDDEN = 256
MSG = 64
N_CORES = 8
NPC = N_NODES // N_CORES  # nodes per core
WIN = 16  # aggregation window (nodes per sel matmul)
NCHUNK = 128  # node chunk (psum accumulator rows)
GRP = 500  # gru node-group free dim

F32 = np.float32
BF16_SCRATCH = None


def _to_bf16_bits(a32):
    """fp32 -> bf16 (round-to-nearest-even) kept as float32 values."""
    a = np.asarray(a32, dtype=np.float32)
    u = a.view(np.uint32)
    rounded = ((u + 0x7FFF + ((u >> 16) & 1)) & 0xFFFF0000).astype(np.uint32)
    return rounded.view(np.float32)


# --------------------------------------------------------------------------
# host preprocessing
# --------------------------------------------------------------------------

def host_prep(x, h, W_enc, b_enc, W_dec, b_dec, W_ih, b_ih, W_hh, b_hh,
              gumbel, src, dst):
    src = np.asarray(src).astype(np.int32)
    dst = np.asarray(dst).astype(np.int32)
    x = np.asarray(x, F32)
    h = np.asarray(h, F32)
    gumbel = np.asarray(gumbel, F32)

    n = x.shape[0]
    npc = n // N_CORES

    # ---- per-core relabeling + edge lists ----
    cores = []
    for c in range(N_CORES):
        lo, hi = c * npc, (c + 1) * npc
        eids = np.nonzero((dst >= lo) & (dst < hi))[0]
        dloc = dst[eids] - lo
        deg = np.bincount(dloc, minlength=npc)
        perm = np.argsort(-deg, kind="stable")  # relabeled k -> orig local node
        # edges grouped by dst node
        order = np.argsort(dloc, kind="stable")
        eids_sorted = eids[order]
        starts = np.zeros(npc + 1, dtype=np.int64)
        np.cumsum(deg, out=starts[1:])
        cores.append(dict(perm=perm, deg=deg, eids=eids_sorted, starts=starts))

    # ---- shared slot schedule ----
    degs_sorted = np.stack([c["deg"][c["perm"]] for c in cores])  # [8, npc] desc
    D = degs_sorted.max(axis=0).astype(np.int64)  # shared slots per rank

    slot_node = []  # relabeled node id per slot, -1 pad
    blk_woff = []
    blk_chunk = []
    fill = 0
    woff = -1
    bchunk = -1

    def close_block():
        nonlocal fill, woff, bchunk
        if woff < 0:
            return
        pad = (-fill) % 128
        slot_node.extend([-1] * pad)
        fill = 0
        woff = -1
        bchunk = -1

    for k in range(npc):
        chunk = k // NCHUNK
        for _ in range(D[k]):
            if woff >= 0 and (k >= woff + WIN or chunk != bchunk):
                close_block()
            if woff < 0:
                woff = min(k, chunk * NCHUNK + NCHUNK - WIN)
                bchunk = chunk
                blk_woff.append(woff)
                blk_chunk.append(chunk)
            slot_node.append(k)
            fill += 1
            if fill == 128:
                fill = 0
                woff = -1
                bchunk = -1
    close_block()

    slot_node = np.asarray(slot_node, dtype=np.int64)
    nblk = len(blk_woff)
    L = nblk * 128
    assert slot_node.shape[0] == L
    blk_woff = np.asarray(blk_woff, dtype=np.int64)
    blk_chunk = np.asarray(blk_chunk, dtype=np.int64)

    # per-chunk start/stop block index
    nchunks = (npc + NCHUNK - 1) // NCHUNK
    chunk_first = np.full(nchunks, -1, dtype=np.int64)
    chunk_last = np.full(nchunks, -1, dtype=np.int64)
    for b in range(nblk):
        ck = blk_chunk[b]
        if chunk_first[ck] < 0:
            chunk_first[ck] = b
        chunk_last[ck] = b

    # slot -> position: position (p, b): slot = b*128 + p
    # per-slot sel column (or -1)
    sel_col = np.where(slot_node >= 0,
                       slot_node - np.repeat(blk_woff, 128),
                       -1)

    # per-slot j-th-in-edge rank within its node (shared). slot_node is
    # non-decreasing over valid slots, so per-node ranks are just aranges.
    slot_rank = np.zeros(L, dtype=np.int64)
    valid = slot_node >= 0
    sn = slot_node[valid]
    if len(sn):
        slot_rank[valid] = np.concatenate(
            [np.arange(cnt) for cnt in np.bincount(sn, minlength=npc)])

    sched = dict(nblk=nblk, L=L, blk_woff=blk_woff, blk_chunk=blk_chunk,
                 chunk_first=chunk_first, chunk_last=chunk_last,
                 slot_node=slot_node, sel_col=sel_col, slot_rank=slot_rank,
                 nchunks=nchunks, npc=npc)

    # global node id -> row in the (relabeled-order) allgathered zdiff table
    glob2row = np.zeros(n, dtype=np.int32)
    for c in range(N_CORES):
        inv = np.empty(npc, dtype=np.int64)
        inv[cores[c]["perm"]] = np.arange(npc)
        glob2row[c * npc:(c + 1) * npc] = (c * npc + inv).astype(np.int32)

    # ---- per-core device arrays ----
    per_core = []

    # shared weight layouts (compile-time constant folding, O(params) work)
    W_enc = np.asarray(W_enc, F32)
    b_enc = np.asarray(b_enc, F32)
    W_dec = np.asarray(W_dec, F32)
    WdiffT = np.ascontiguousarray((W_enc[0::2] - W_enc[1::2]).T)   # [512, 64]
    bdiff_row = (b_enc[0::2] - b_enc[1::2]).reshape(1, MSG)        # [1, 64]
    WddT = np.ascontiguousarray((W_dec[:, 0::2] - W_dec[:, 1::2]).T)  # [64,128]
    wodd_row = W_dec[:, 1::2].sum(axis=1).reshape(1, 2 * MSG)      # [1, 128]
    bdec_col = np.asarray(b_dec, F32).reshape(2 * MSG, 1)          # [128,1]
    WihT = np.ascontiguousarray(np.asarray(W_ih, F32).T)           # [384,768]
    WhhT = np.ascontiguousarray(np.asarray(W_hh, F32).T)           # [256,768]
    bih6 = np.ascontiguousarray(np.asarray(b_ih, F32).reshape(6, 128).T)  # [128,6]
    bhh6 = np.ascontiguousarray(np.asarray(b_hh, F32).reshape(6, 128).T)  # [128,6]

    for c in range(N_CORES):
        cd = cores[c]
        perm = cd["perm"]
        lo = c * npc
        # slot -> edge id (or -1): node k rank r -> edge if r < deg
        node_orig = np.where(slot_node >= 0, perm[np.clip(slot_node, 0, None)], 0)
        degs = cd["deg"][node_orig]
        real = valid & (slot_rank < degs)
        edge_of_slot = np.full(L, -1, dtype=np.int64)
        idx_in_sorted = cd["starts"][node_orig] + slot_rank
        edge_of_slot[real] = cd["eids"][np.clip(idx_in_sorted, 0,
                                                len(cd["eids"]) - 1)][real]

        e = edge_of_slot
        er = np.clip(e, 0, None)
        gidx_s = np.where(e >= 0, glob2row[src[er]], 0).astype(np.int16)
        G0_s = np.where((e >= 0)[:, None], -gumbel[er, :, 0], F32(0))  # negated
        G1_s = np.where((e >= 0)[:, None], gumbel[er, :, 1], F32(0))
        sel_s = np.zeros((L, WIN), dtype=np.float32)
        rs = np.nonzero(real)[0]
        sel_s[rs, sel_col[rs]] = 1.0

        # position (p, b) = slot b*128+p  ->  array[p, b] = slot-major transpose
        def pos(a):
            return np.ascontiguousarray(
                a.reshape(nblk, 128, *a.shape[1:]).swapaxes(0, 1))

        xi = x[lo + perm]  # [npc, 256] relabeled
        hi_ = h[lo + perm]
        xhT = np.ascontiguousarray(
            np.concatenate([xi, hi_], axis=1).T)  # [512, npc]
        deg_row = cd["deg"][perm].astype(F32).reshape(1, npc)  # [1, npc]

        # dma_gather index layout: index i lives at [i % 16, i // 16],
        # replicated 8x across partition groups of 16 (one per Q7 core)
        gidx16 = np.zeros((16, L // 16), dtype=np.int16)
        ii = np.arange(L)
        gidx16[ii % 16, ii // 16] = gidx_s
        gidx16 = np.tile(gidx16, (8, 1))  # [128, L//16]

        per_core.append(dict(
            xhT=xhT,
            WdiffT=WdiffT, bdiff=bdiff_row, WddT=WddT, wodd=wodd_row,
            bdec=bdec_col, WihT=WihT, WhhT=WhhT, bih6=bih6, bhh6=bhh6,
            G0=pos(G0_s), G1=pos(G1_s),
            gidx=pos(gidx_s.astype(np.int32)),  # for the numpy golden model
            gidx16=gidx16, sel=pos(sel_s).astype(np.float32),
            deg=deg_row,
        ))

    return sched, per_core, cores


def host_post(results, cores):
    """results: list of dicts with 'outT' [256, npc] per core."""
    out = np.zeros((N_CORES * NPC, HIDDEN), dtype=np.float32)
    for c in range(N_CORES):
        perm = cores[c]["perm"]
        outT = np.asarray(results[c]["outT"])  # [256, npc]
        out[c * NPC + perm, :] = outT.T
    return out


# --------------------------------------------------------------------------
# numpy golden model of the device program (for validation)
# --------------------------------------------------------------------------

def golden_run(sched, per_core):
    npc = sched["npc"]
    nblk = sched["nblk"]
    results = []
    # phase 1: zdiff slices (fp32) + allgather
    zslices = []
    for c in range(N_CORES):
        d = per_core[c]
        z = d["xhT"].T @ d["WdiffT"] + d["bdiff"]  # [npc, 64]
        zslices.append(z.astype(np.float32))
    zfull = np.concatenate(zslices, axis=0)  # [20000, 64]

    for c in range(N_CORES):
        d = per_core[c]
        # phase 2: bits (fp32); G0 is pre-negated so B = g1 - g0
        B = (d["G1"] + d["G0"]).astype(np.float32)
        bits = (zfull[d["gidx"]] >= B).astype(np.float32)  # [128, nblk, 64]
        # aggregation
        cnt = np.zeros((npc + WIN, MSG), dtype=np.float32)
        for b in range(nblk):
            woff = sched["blk_woff"][b]
            contrib = d["sel"][:, b, :].T @ bits[:, b, :]  # [WIN, 64]
            cnt[woff:woff + WIN, :] += contrib
        cnt = cnt[:npc]
        deg = d["deg"][0]  # [npc]
        # phase 3: GRU (fp32); dec via folded W_dec
        decT = (d["WddT"].T @ cnt.T + d["wodd"].T @ d["deg"]
                + d["bdec"])  # [128, npc]
        xT = d["xhT"][:256]
        hT = d["xhT"][256:]
        inpT = np.concatenate([xT, decT], axis=0)  # [384, npc]
        gi = d["WihT"].T @ inpT + np.concatenate(
            [d["bih6"][:, i] for i in range(6)])[:, None]
        gh = d["WhhT"].T @ hT + np.concatenate(
            [d["bhh6"][:, i] for i in range(6)])[:, None]
        r = 1.0 / (1.0 + np.exp(-(gi[0:256] + gh[0:256])))
        zg = 1.0 / (1.0 + np.exp(-(gi[256:512] + gh[256:512])))
        ng = np.tanh(gi[512:768] + r * gh[512:768])
        outT = ng + zg * (hT - ng)
        results.append(dict(outT=outT.astype(np.float32)))
    return results


# --------------------------------------------------------------------------
# bass kernel
# --------------------------------------------------------------------------

CHB = 32  # blocks per edge-phase DMA chunk


def build_nc(sched, debug_taps=False):
    import concourse.bass as bass
    import concourse.mybir as mybir
    from concourse import bacc, tile
    from contextlib import ExitStack

    dt = mybir.dt
    f32 = dt.float32
    bf16 = dt.bfloat16
    i32 = dt.int32
    Alu = mybir.AluOpType
    Act = mybir.ActivationFunctionType

    npc = sched["npc"]
    nblk = sched["nblk"]
    nchunks = sched["nchunks"]
    blk_woff = sched["blk_woff"]
    blk_chunk = sched["blk_chunk"]
    chunk_first = sched["chunk_first"]
    chunk_last = sched["chunk_last"]
    assert chunk_first.min() >= 0, "empty node chunk unsupported"
    ntile = (npc + 127) // 128  # zdiff node tiles

    nc = bacc.Bacc(None, num_devices=N_CORES)

    def din(name, shape, dtype):
        return nc.dram_tensor(name, shape, dtype, kind="ExternalInput")

    xhT_d = din("xhT", [512, npc], f32)
    WdiffT_d = din("WdiffT", [512, MSG], f32)
    bdiff_d = din("bdiff", [1, MSG], f32)
    WddT_d = din("WddT", [MSG, 128], f32)
    wodd_d = din("wodd", [1, 128], f32)
    bdec_d = din("bdec", [128, 1], f32)
    WihT_d = din("WihT", [384, 768], f32)
    WhhT_d = din("WhhT", [256, 768], f32)
    bih6_d = din("bih6", [128, 6], f32)
    bhh6_d = din("bhh6", [128, 6], f32)
    G0_d = din("G0", [128, nblk, MSG], f32)
    G1_d = din("G1", [128, nblk, MSG], f32)
    gidx16_d = din("gidx16", [128, nblk * 8], dt.int16)
    sel_d = din("sel", [128, nblk, WIN], bf16)
    deg_d = din("deg", [1, npc], f32)
    outT_d = nc.dram_tensor("outT", [256, npc], f32, kind="ExternalOutput")
    if debug_taps:
        dbgz_d = nc.dram_tensor("dbg_zfull", [N_CORES * npc, MSG], f32,
                                kind="ExternalOutput")
        dbgc_d = nc.dram_tensor("dbg_cnt", [MSG, npc], f32,
                                kind="ExternalOutput")

    with tile.TileContext(nc) as tc, ExitStack() as ctx:
        const = ctx.enter_context(tc.tile_pool(name="const", bufs=1))
        dram = ctx.enter_context(tc.tile_pool(name="dram", bufs=1, space="DRAM"))

        # ---- persistent loads ----
        def load(name, dten, shape, dtype, src_ap=None):
            t = const.tile(shape, dtype, tag=name)
            nc.sync.dma_start(t[:], dten[:] if src_ap is None else src_ap)
            return t

        xh = [load(f"xh{k}", None, [128, npc], f32,
                   src_ap=xhT_d[128 * k:128 * (k + 1), :]) for k in range(4)]
        Wd = [load(f"Wd{k}", None, [128, MSG], f32,
                   src_ap=WdiffT_d[128 * k:128 * (k + 1), :]) for k in range(4)]
        Wih = [load(f"Wih{k}", None, [128, 768], f32,
                    src_ap=WihT_d[128 * k:128 * (k + 1), :]) for k in range(3)]
        Whh = [load(f"Whh{k}", None, [128, 768], f32,
                    src_ap=WhhT_d[128 * k:128 * (k + 1), :]) for k in range(2)]
        bdiff_t = load("bdiff", bdiff_d, [1, MSG], f32)
        Wdd_t = load("Wdd", WddT_d, [MSG, 128], f32)
        wodd_t = load("wodd", wodd_d, [1, 128], f32)
        bdec_t = load("bdec", bdec_d, [128, 1], f32)
        bih6_t = load("bih6", bih6_d, [128, 6], f32)
        bhh6_t = load("bhh6", bhh6_d, [128, 6], f32)
        gidx_t = load("gidx16", gidx16_d, [128, nblk * 8], dt.int16)
        sel_t = load("sel", sel_d, [128, nblk, WIN], bf16)
        deg_t = load("deg", deg_d, [1, npc], f32)

        ones_t = const.tile([1, npc], f32, tag="ones")
        nc.vector.memset(ones_t[:], 1.0)
        zerot = const.tile([128, 128], bf16, tag="zerot")
        nc.vector.memset(zerot[:], 0.0)
        bsum_t = const.tile([128, 6], f32, tag="bsum")
        nc.vector.tensor_add(bsum_t[:], bih6_t[:], bhh6_t[:])
        cntT = const.tile([MSG, npc], f32, tag="cntT")

        zslice_dt = dram.tile([npc, MSG], f32, tag="zslice")
        zfull_dt = dram.tile([N_CORES * npc, MSG], f32, tag="zfull")

        # ---- phase 1: zdiff slice + allgather ----
        with tc.tile_pool(name="pz", bufs=2, space="PSUM") as pzp, \
                tc.tile_pool(name="zs", bufs=3) as zsp:
            for t in range(ntile):
                c0 = t * 128
                p = min(128, npc - c0)
                pz = pzp.tile([128, MSG], f32, tag="pz")
                for k in range(4):
                    nc.tensor.matmul(pz[:p, :], xh[k][:, c0:c0 + p], Wd[k][:],
                                     start=(k == 0), stop=False)
                nc.tensor.matmul(pz[:p, :], ones_t[:, c0:c0 + p], bdiff_t[:],
                                 start=False, stop=True)
                zs = zsp.tile([128, MSG], f32, tag="zs")
                nc.scalar.activation(zs[:p, :], pz[:p, :], Act.Identity)
                nc.sync.dma_start(zslice_dt[c0:c0 + p, :], zs[:p, :])

        nc.gpsimd.collective_compute(
            "AllGather", Alu.bypass,
            replica_groups=[list(range(N_CORES))],
            ins=[zslice_dt.opt()], outs=[zfull_dt.opt()],
        )

        # ---- phase 2: edge bits + count aggregation ----
        nechunks = (nblk + CHB - 1) // CHB
        pcnt = None
        with tc.tile_pool(name="ed", bufs=2) as edp, \
                tc.tile_pool(name="pc", bufs=2, space="PSUM") as pcp:
            for ci in range(nechunks):
                b0 = ci * CHB
                b1 = min(nblk, b0 + CHB)
                w = b1 - b0
                A = edp.tile([128, w, MSG], f32, tag="A")
                B = edp.tile([128, w, MSG], f32, tag="B")
                # B = g1 - g0  (G0 input is pre-negated; CCE add during DMA)
                nc.sync.dma_start(B[:], G1_d[:, b0:b1, :])
                nc.gpsimd.dma_start(B[:], G0_d[:, b0:b1, :], accum_op=Alu.add)
                # A = zdiff[src[slot]] -- in sub-gathers of <=1024 indices
                # (larger single dma_gather calls fault the exec unit)
                for j0 in range(0, w, 8):
                    j1 = min(w, j0 + 8)
                    nc.gpsimd.dma_gather(
                        out_ap=A[:, j0:j1, :], in_ap=zfull_dt[:],
                        idxs_ap=gidx_t[:, (b0 + j0) * 8:(b0 + j1) * 8],
                        num_idxs=(j1 - j0) * 128,
                        num_idxs_reg=(j1 - j0) * 128,
                        elem_size=MSG,
                    )
                bits = edp.tile([128, w, MSG], bf16, tag="bits")
                nc.vector.tensor_tensor(bits[:], A[:], B[:], op=Alu.is_ge)

                for b in range(b0, b1):
                    ck = int(blk_chunk[b])
                    wl = int(blk_woff[b]) - ck * NCHUNK
                    if b == chunk_first[ck]:
                        # sym-major count accumulator [64 syms, 128 nodes]
                        pcnt = pcp.tile([MSG, NCHUNK], f32, tag="pcnt")
                        nc.tensor.matmul(pcnt[:], zerot[:, :MSG], zerot[:],
                                         start=True, stop=False,
                                         skip_group_check=True)
                    nc.tensor.matmul(
                        pcnt[:, wl:wl + WIN], bits[:, b - b0, :], sel_t[:, b, :],
                        start=False, stop=(b == chunk_last[ck]),
                        skip_group_check=True)
                    if b == chunk_last[ck]:
                        p = min(128, npc - ck * NCHUNK)
                        nc.scalar.activation(
                            cntT[:, ck * NCHUNK:ck * NCHUNK + p], pcnt[:, :p],
                            Act.Identity)

        if debug_taps:
            nc.sync.dma_start(dbgz_d[:], zfull_dt[:])
            nc.sync.dma_start(dbgc_d[:], cntT[:])

        # ---- phase 3: GRU ----
        ngrp = (npc + GRP - 1) // GRP
        with tc.tile_pool(name="pg", bufs=2, space="PSUM") as pgp, \
                tc.tile_pool(name="pn", bufs=2, space="PSUM") as pnp, \
                tc.tile_pool(name="gr", bufs=2) as grp_:
            for g in range(ngrp):
                c0 = g * GRP
                n = min(GRP, npc - c0)
                sl = slice(c0, c0 + n)

                pd = pgp.tile([128, GRP], f32, tag="pd")
                nc.tensor.matmul(pd[:, :n], Wdd_t[:], cntT[:, sl],
                                 start=True, stop=False)
                nc.tensor.matmul(pd[:, :n], wodd_t[:], deg_t[:, sl],
                                 start=False, stop=True)
                decT = grp_.tile([128, GRP], f32, tag="dec")
                nc.scalar.activation(decT[:, :n], pd[:, :n], Act.Identity,
                                     bias=bdec_t[:])

                gates = []
                for f in range(4):  # r0, r1, z0, z1
                    pg = pgp.tile([128, GRP], f32, tag="pg")
                    fs = slice(f * 128, (f + 1) * 128)
                    nc.tensor.matmul(pg[:, :n], Wih[0][:, fs], xh[0][:, sl],
                                     start=True, stop=False)
                    nc.tensor.matmul(pg[:, :n], Wih[1][:, fs], xh[1][:, sl],
                                     start=False, stop=False)
                    nc.tensor.matmul(pg[:, :n], Wih[2][:, fs], decT[:, :n],
                                     start=False, stop=False)
                    nc.tensor.matmul(pg[:, :n], Whh[0][:, fs], xh[2][:, sl],
                                     start=False, stop=False)
                    nc.tensor.matmul(pg[:, :n], Whh[1][:, fs], xh[3][:, sl],
                                     start=False, stop=True)
                    gt = grp_.tile([128, GRP], f32, tag=f"gate{f}")
                    nc.scalar.activation(gt[:, :n], pg[:, :n], Act.Sigmoid,
                                         bias=bsum_t[:, f:f + 1])
                    gates.append(gt)

                for f2 in range(2):  # n-gate + output, 128-row halves
                    fs = slice((4 + f2) * 128, (5 + f2) * 128)
                    pgin = pnp.tile([128, GRP], f32, tag="pgin")
                    nc.tensor.matmul(pgin[:, :n], Wih[0][:, fs], xh[0][:, sl],
                                     start=True, stop=False)
                    nc.tensor.matmul(pgin[:, :n], Wih[1][:, fs], xh[1][:, sl],
                                     start=False, stop=False)
                    nc.tensor.matmul(pgin[:, :n], Wih[2][:, fs], decT[:, :n],
                                     start=False, stop=True)
                    pghn = pnp.tile([128, GRP], f32, tag="pghn")
                    nc.tensor.matmul(pghn[:, :n], Whh[0][:, fs], xh[2][:, sl],
                                     start=True, stop=False)
                    nc.tensor.matmul(pghn[:, :n], Whh[1][:, fs], xh[3][:, sl],
                                     start=False, stop=True)

                    hn = grp_.tile([128, GRP], f32, tag="hn")
                    nc.scalar.activation(hn[:, :n], pghn[:, :n], Act.Identity,
                                         bias=bhh6_t[:, 4 + f2:5 + f2])
                    t1 = grp_.tile([128, GRP], f32, tag="t1")
                    nc.vector.tensor_mul(t1[:, :n], gates[f2][:, :n], hn[:, :n])
                    t2 = grp_.tile([128, GRP], f32, tag="t2")
                    nc.vector.tensor_add(t2[:, :n], t1[:, :n], pgin[:, :n])
                    ng = grp_.tile([128, GRP], f32, tag="ng")
                    nc.scalar.activation(ng[:, :n], t2[:, :n], Act.Tanh,
                                         bias=bih6_t[:, 4 + f2:5 + f2])
                    u = grp_.tile([128, GRP], f32, tag="u")
                    nc.vector.tensor_sub(u[:, :n], xh[2 + f2][:, sl], ng[:, :n])
                    v = grp_.tile([128, GRP], f32, tag="v")
                    nc.vector.tensor_mul(v[:, :n], gates[2 + f2][:, :n], u[:, :n])
                    o = grp_.tile([128, GRP], f32, tag="o")
                    nc.vector.tensor_add(o[:, :n], ng[:, :n], v[:, :n])
                    nc.sync.dma_start(outT_d[f2 * 128:(f2 + 1) * 128, sl],
                                      o[:, :n])

    return nc


def run_on_hw(sched, per_core, trace=False, debug_taps=False):
    from concourse.bass_utils import run_bass_kernel_spmd
    import ml_dtypes

    nc = build_nc(sched, debug_taps=debug_taps)
    nc.compile()
    in_maps = []
    for d in per_core:
        m = {}
        for k, v in d.items():
            if k == "gidx":
                continue  # golden-model only
            if k == "sel":
                m[k] = np.ascontiguousarray(v.astype(ml_dtypes.bfloat16))
            elif k == "gidx16":
                m[k] = np.ascontiguousarray(v.astype(np.int16))
            else:
                m[k] = np.ascontiguousarray(v.astype(np.float32))
        in_maps.append(m)
    res = run_bass_kernel_spmd(nc, in_maps, list(range(N_CORES)), trace=trace)
    return res


def kernel(**inputs):
    sched, per_core, cores = host_prep(**inputs)
    res = run_on_hw(sched, per_core)
    return host_post(res.results, cores)


if __name__ == "__main__":
    pass


# revision 37
# speedup vs baseline: 1.0552x; 1.0552x over previous
"""Trainium2 Bass kernel for nn_DiscreteComm (GNN message passing).

Algorithm (matches the reference AS EXECUTED ON THIS JAX/NEURON STACK, where
jax.ops.segment_max lowers to segment-SUM with 0 identity -- verified):
  - m = y_hard + y_soft - stop_grad(y_soft) == y_hard exactly in fp arithmetic,
    so each edge contributes one BIT per message symbol:
        bit[e,j] = (zdiff[src[e],j] + g0[e,j] >= g1[e,j])
    where zdiff[v,j] = (x[v]||h[v]) . (W_enc[2j]-W_enc[2j+1]) + (b_enc[2j]-b_enc[2j+1])
  - c[v,2j] = cnt[v,j] = sum of bits over in-edges;  c[v,2j+1] = deg[v]-cnt[v,j]
  - dec = W_dec @ c + b_dec  folds to  Wdd @ cnt + w_odd (x) deg + b_dec,
    Wdd = W_dec[:,even]-W_dec[:,odd], w_odd = sum_j W_dec[:,2j+1].
  - GRU update per node (dense GEMMs, fp32 throughout; bit decisions fp32).

Distribution: edges sharded by dst range (2500 nodes/core, 8 cores). Each core
computes zdiff for its own nodes; AllGather makes the full fp32 zdiff table;
per-edge bits via indirect-DMA gather (fused +G0 via CCE add) + one DVE is_ge;
per-node counts via tiny one-hot `sel` matmuls into PSUM windows; GRU in fp32.

SPMD note: all 8 cores run one program, so the block schedule (node windows,
chunk boundaries) is shared. Per-core in-degree sequences are aligned by
relabeling each core's nodes in descending-degree order; shared slot counts
are the per-rank max over cores (~1% padding).
"""

import os
import sys

import numpy as np

sys.path.insert(0, "/opt/trn_rl_repo")

N_NODES = 20000
N_EDGES = 640000
HIDDEN = 256
MSG = 64
N_CORES = 8
NPC = N_NODES // N_CORES  # nodes per core
WIN = 16  # aggregation window (nodes per sel matmul)
NCHUNK = 128  # node chunk (psum accumulator rows)
GRP = 500  # gru node-group free dim

F32 = np.float32
BF16_SCRATCH = None


def _to_bf16_bits(a32):
    """fp32 -> bf16 (round-to-nearest-even) kept as float32 values."""
    a = np.asarray(a32, dtype=np.float32)
    u = a.view(np.uint32)
    rounded = ((u + 0x7FFF + ((u >> 16) & 1)) & 0xFFFF0000).astype(np.uint32)
    return rounded.view(np.float32)


# --------------------------------------------------------------------------
# host preprocessing
# --------------------------------------------------------------------------

def host_prep(x, h, W_enc, b_enc, W_dec, b_dec, W_ih, b_ih, W_hh, b_hh,
              gumbel, src, dst):
    src = np.asarray(src).astype(np.int32)
    dst = np.asarray(dst).astype(np.int32)
    x = np.asarray(x, F32)
    h = np.asarray(h, F32)
    gumbel = np.asarray(gumbel, F32)

    n = x.shape[0]
    npc = n // N_CORES

    # ---- per-core relabeling + edge lists ----
    cores = []
    for c in range(N_CORES):
        lo, hi = c * npc, (c + 1) * npc
        eids = np.nonzero((dst >= lo) & (dst < hi))[0]
        dloc = dst[eids] - lo
        deg = np.bincount(dloc, minlength=npc)
        perm = np.argsort(-deg, kind="stable")  # relabeled k -> orig local node
        # edges grouped by dst node
        order = np.argsort(dloc, kind="stable")
        eids_sorted = eids[order]
        starts = np.zeros(npc + 1, dtype=np.int64)
        np.cumsum(deg, out=starts[1:])
        cores.append(dict(perm=perm, deg=deg, eids=eids_sorted, starts=starts))

    # ---- shared slot schedule ----
    degs_sorted = np.stack([c["deg"][c["perm"]] for c in cores])  # [8, npc] desc
    D = degs_sorted.max(axis=0).astype(np.int64)  # shared slots per rank

    slot_node = []  # relabeled node id per slot, -1 pad
    blk_woff = []
    blk_chunk = []
    fill = 0
    woff = -1
    bchunk = -1

    def close_block():
        nonlocal fill, woff, bchunk
        if woff < 0:
            return
        pad = (-fill) % 128
        slot_node.extend([-1] * pad)
        fill = 0
        woff = -1
        bchunk = -1

    for k in range(npc):
        chunk = k // NCHUNK
        for _ in range(D[k]):
            if woff >= 0 and (k >= woff + WIN or chunk != bchunk):
                close_block()
            if woff < 0:
                woff = min(k, chunk * NCHUNK + NCHUNK - WIN)
                bchunk = chunk
                blk_woff.append(woff)
                blk_chunk.append(chunk)
            slot_node.append(k)
            fill += 1
            if fill == 128:
                fill = 0
                woff = -1
                bchunk = -1
    close_block()

    slot_node = np.asarray(slot_node, dtype=np.int64)
    nblk = len(blk_woff)
    L = nblk * 128
    assert slot_node.shape[0] == L
    blk_woff = np.asarray(blk_woff, dtype=np.int64)
    blk_chunk = np.asarray(blk_chunk, dtype=np.int64)

    # per-chunk start/stop block index
    nchunks = (npc + NCHUNK - 1) // NCHUNK
    chunk_first = np.full(nchunks, -1, dtype=np.int64)
    chunk_last = np.full(nchunks, -1, dtype=np.int64)
    for b in range(nblk):
        ck = blk_chunk[b]
        if chunk_first[ck] < 0:
            chunk_first[ck] = b
        chunk_last[ck] = b

    # slot -> position: position (p, b): slot = b*128 + p
    # per-slot sel column (or -1)
    sel_col = np.where(slot_node >= 0,
                       slot_node - np.repeat(blk_woff, 128),
                       -1)

    # per-slot j-th-in-edge rank within its node (shared). slot_node is
    # non-decreasing over valid slots, so per-node ranks are just aranges.
    slot_rank = np.zeros(L, dtype=np.int64)
    valid = slot_node >= 0
    sn = slot_node[valid]
    if len(sn):
        slot_rank[valid] = np.concatenate(
            [np.arange(cnt) for cnt in np.bincount(sn, minlength=npc)])

    sched = dict(nblk=nblk, L=L, blk_woff=blk_woff, blk_chunk=blk_chunk,
                 chunk_first=chunk_first, chunk_last=chunk_last,
                 slot_node=slot_node, sel_col=sel_col, slot_rank=slot_rank,
                 nchunks=nchunks, npc=npc)

    # global node id -> row in the (relabeled-order) allgathered zdiff table
    glob2row = np.zeros(n, dtype=np.int32)
    for c in range(N_CORES):
        inv = np.empty(npc, dtype=np.int64)
        inv[cores[c]["perm"]] = np.arange(npc)
        glob2row[c * npc:(c + 1) * npc] = (c * npc + inv).astype(np.int32)

    # ---- per-core device arrays ----
    per_core = []

    # shared weight layouts (compile-time constant folding, O(params) work)
    W_enc = np.asarray(W_enc, F32)
    b_enc = np.asarray(b_enc, F32)
    W_dec = np.asarray(W_dec, F32)
    WdiffT = np.ascontiguousarray((W_enc[0::2] - W_enc[1::2]).T)   # [512, 64]
    bdiff_row = (b_enc[0::2] - b_enc[1::2]).reshape(1, MSG)        # [1, 64]
    WddT = np.ascontiguousarray((W_dec[:, 0::2] - W_dec[:, 1::2]).T)  # [64,128]
    wodd_row = W_dec[:, 1::2].sum(axis=1).reshape(1, 2 * MSG)      # [1, 128]
    bdec_col = np.asarray(b_dec, F32).reshape(2 * MSG, 1)          # [128,1]
    WihT = np.ascontiguousarray(np.asarray(W_ih, F32).T)           # [384,768]
    WhhT = np.ascontiguousarray(np.asarray(W_hh, F32).T)           # [256,768]
    bih6 = np.ascontiguousarray(np.asarray(b_ih, F32).reshape(6, 128).T)  # [128,6]
    bhh6 = np.ascontiguousarray(np.asarray(b_hh, F32).reshape(6, 128).T)  # [128,6]

    for c in range(N_CORES):
        cd = cores[c]
        perm = cd["perm"]
        lo = c * npc
        # slot -> edge id (or -1): node k rank r -> edge if r < deg
        node_orig = np.where(slot_node >= 0, perm[np.clip(slot_node, 0, None)], 0)
        degs = cd["deg"][node_orig]
        real = valid & (slot_rank < degs)
        edge_of_slot = np.full(L, -1, dtype=np.int64)
        idx_in_sorted = cd["starts"][node_orig] + slot_rank
        edge_of_slot[real] = cd["eids"][np.clip(idx_in_sorted, 0,
                                                len(cd["eids"]) - 1)][real]

        e = edge_of_slot
        er = np.clip(e, 0, None)
        gidx_s = np.where(e >= 0, glob2row[src[er]], 0).astype(np.int16)
        G0_s = np.where((e >= 0)[:, None], -gumbel[er, :, 0], F32(0))  # negated
        G1_s = np.where((e >= 0)[:, None], gumbel[er, :, 1], F32(0))
        sel_s = np.zeros((L, WIN), dtype=np.float32)
        rs = np.nonzero(real)[0]
        sel_s[rs, sel_col[rs]] = 1.0

        # position (p, b) = slot b*128+p  ->  array[p, b] = slot-major transpose
        def pos(a):
            return np.ascontiguousarray(
                a.reshape(nblk, 128, *a.shape[1:]).swapaxes(0, 1))

        xi = x[lo + perm]  # [npc, 256] relabeled
        hi_ = h[lo + perm]
        xhT = np.ascontiguousarray(
            np.concatenate([xi, hi_], axis=1).T)  # [512, npc]
        deg_row = cd["deg"][perm].astype(F32).reshape(1, npc)  # [1, npc]

        # dma_gather index layout: index i lives at [i % 16, i // 16],
        # replicated 8x across partition groups of 16 (one per Q7 core)
        gidx16 = np.zeros((16, L // 16), dtype=np.int16)
        ii = np.arange(L)
        gidx16[ii % 16, ii // 16] = gidx_s
        gidx16 = np.tile(gidx16, (8, 1))  # [128, L//16]

        per_core.append(dict(
            xhT=xhT,
            WdiffT=WdiffT, bdiff=bdiff_row, WddT=WddT, wodd=wodd_row,
            bdec=bdec_col, WihT=WihT, WhhT=WhhT, bih6=bih6, bhh6=bhh6,
            G0=pos(G0_s), G1=pos(G1_s),
            gidx=pos(gidx_s.astype(np.int32)),  # for the numpy golden model
            gidx16=gidx16, sel=pos(sel_s).astype(np.float32),
            deg=deg_row,
        ))

    return sched, per_core, cores


def host_post(results, cores):
    """results: list of dicts with 'outT' [256, npc] per core."""
    out = np.zeros((N_CORES * NPC, HIDDEN), dtype=np.float32)
    for c in range(N_CORES):
        perm = cores[c]["perm"]
        outT = np.asarray(results[c]["outT"])  # [256, npc]
        out[c * NPC + perm, :] = outT.T
    return out


# --------------------------------------------------------------------------
# numpy golden model of the device program (for validation)
# --------------------------------------------------------------------------

def golden_run(sched, per_core):
    npc = sched["npc"]
    nblk = sched["nblk"]
    results = []
    # phase 1: zdiff slices (fp32) + allgather
    zslices = []
    for c in range(N_CORES):
        d = per_core[c]
        z = d["xhT"].T @ d["WdiffT"] + d["bdiff"]  # [npc, 64]
        zslices.append(z.astype(np.float32))
    zfull = np.concatenate(zslices, axis=0)  # [20000, 64]

    for c in range(N_CORES):
        d = per_core[c]
        # phase 2: bits (fp32); G0 is pre-negated so B = g1 - g0
        B = (d["G1"] + d["G0"]).astype(np.float32)
        bits = (zfull[d["gidx"]] >= B).astype(np.float32)  # [128, nblk, 64]
        # aggregation
        cnt = np.zeros((npc + WIN, MSG), dtype=np.float32)
        for b in range(nblk):
            woff = sched["blk_woff"][b]
            contrib = d["sel"][:, b, :].T @ bits[:, b, :]  # [WIN, 64]
            cnt[woff:woff + WIN, :] += contrib
        cnt = cnt[:npc]
        deg = d["deg"][0]  # [npc]
        # phase 3: GRU (fp32); dec via folded W_dec
        decT = (d["WddT"].T @ cnt.T + d["wodd"].T @ d["deg"]
                + d["bdec"])  # [128, npc]
        xT = d["xhT"][:256]
        hT = d["xhT"][256:]
        inpT = np.concatenate([xT, decT], axis=0)  # [384, npc]
        gi = d["WihT"].T @ inpT + np.concatenate(
            [d["bih6"][:, i] for i in range(6)])[:, None]
        gh = d["WhhT"].T @ hT + np.concatenate(
            [d["bhh6"][:, i] for i in range(6)])[:, None]
        r = 1.0 / (1.0 + np.exp(-(gi[0:256] + gh[0:256])))
        zg = 1.0 / (1.0 + np.exp(-(gi[256:512] + gh[256:512])))
        ng = np.tanh(gi[512:768] + r * gh[512:768])
        outT = ng + zg * (hT - ng)
        results.append(dict(outT=outT.astype(np.float32)))
    return results


# --------------------------------------------------------------------------
# bass kernel
# --------------------------------------------------------------------------

CHB = 32  # blocks per edge-phase DMA chunk


def build_nc(sched, debug_taps=False, sim_single_core=False):
    import concourse.bass as bass
    import concourse.mybir as mybir
    from concourse import bacc, tile
    from contextlib import ExitStack

    dt = mybir.dt
    f32 = dt.float32
    bf16 = dt.bfloat16
    i32 = dt.int32
    Alu = mybir.AluOpType
    Act = mybir.ActivationFunctionType

    npc = sched["npc"]
    nblk = sched["nblk"]
    nchunks = sched["nchunks"]
    blk_woff = sched["blk_woff"]
    blk_chunk = sched["blk_chunk"]
    chunk_first = sched["chunk_first"]
    chunk_last = sched["chunk_last"]
    assert chunk_first.min() >= 0, "empty node chunk unsupported"
    ntile = (npc + 127) // 128  # zdiff node tiles

    nc = bacc.Bacc(None, num_devices=N_CORES)

    def din(name, shape, dtype):
        return nc.dram_tensor(name, shape, dtype, kind="ExternalInput")

    xhT_d = din("xhT", [512, npc], f32)
    WdiffT_d = din("WdiffT", [512, MSG], f32)
    bdiff_d = din("bdiff", [1, MSG], f32)
    WddT_d = din("WddT", [MSG, 128], f32)
    wodd_d = din("wodd", [1, 128], f32)
    bdec_d = din("bdec", [128, 1], f32)
    WihT_d = din("WihT", [384, 768], f32)
    WhhT_d = din("WhhT", [256, 768], f32)
    bih6_d = din("bih6", [128, 6], f32)
    bhh6_d = din("bhh6", [128, 6], f32)
    G0_d = din("G0", [128, nblk, MSG], f32)
    G1_d = din("G1", [128, nblk, MSG], f32)
    gidx16_d = din("gidx16", [128, nblk * 8], dt.int16)
    sel_d = din("sel", [128, nblk, WIN], bf16)
    deg_d = din("deg", [1, npc], f32)
    outT_d = nc.dram_tensor("outT", [256, npc], f32, kind="ExternalOutput")
    if debug_taps:
        dbgz_d = nc.dram_tensor("dbg_zfull", [N_CORES * npc, MSG], f32,
                                kind="ExternalOutput")
        dbgc_d = nc.dram_tensor("dbg_cnt", [MSG, npc], f32,
                                kind="ExternalOutput")

    with tile.TileContext(nc) as tc, ExitStack() as ctx:
        const = ctx.enter_context(tc.tile_pool(name="const", bufs=1))
        dram = ctx.enter_context(tc.tile_pool(name="dram", bufs=1, space="DRAM"))

        # ---- persistent loads ----
        _ld_flip = [0]

        def load(name, dten, shape, dtype, src_ap=None):
            t = const.tile(shape, dtype, tag=name)
            eng = nc.sync if _ld_flip[0] % 2 == 0 else nc.scalar
            _ld_flip[0] += 1
            eng.dma_start(t[:], dten[:] if src_ap is None else src_ap)
            return t

        xh = [load(f"xh{k}", None, [128, npc], f32,
                   src_ap=xhT_d[128 * k:128 * (k + 1), :]) for k in range(4)]
        Wd = [load(f"Wd{k}", None, [128, MSG], f32,
                   src_ap=WdiffT_d[128 * k:128 * (k + 1), :]) for k in range(4)]
        Wih = [load(f"Wih{k}", None, [128, 768], f32,
                    src_ap=WihT_d[128 * k:128 * (k + 1), :]) for k in range(3)]
        Whh = [load(f"Whh{k}", None, [128, 768], f32,
                    src_ap=WhhT_d[128 * k:128 * (k + 1), :]) for k in range(2)]
        bdiff_t = load("bdiff", bdiff_d, [1, MSG], f32)
        Wdd_t = load("Wdd", WddT_d, [MSG, 128], f32)
        wodd_t = load("wodd", wodd_d, [1, 128], f32)
        bdec_t = load("bdec", bdec_d, [128, 1], f32)
        bih6_t = load("bih6", bih6_d, [128, 6], f32)
        bhh6_t = load("bhh6", bhh6_d, [128, 6], f32)
        gidx_t = load("gidx16", gidx16_d, [128, nblk * 8], dt.int16)
        sel_t = load("sel", sel_d, [128, nblk, WIN], bf16)
        deg_t = load("deg", deg_d, [1, npc], f32)

        ones_t = const.tile([1, npc], f32, tag="ones")
        nc.vector.memset(ones_t[:], 1.0)
        zerot = const.tile([128, 128], bf16, tag="zerot")
        nc.vector.memset(zerot[:], 0.0)
        bsum_t = const.tile([128, 6], f32, tag="bsum")
        nc.vector.tensor_add(bsum_t[:], bih6_t[:], bhh6_t[:])
        cntT = const.tile([MSG, npc], f32, tag="cntT")

        zslice_dt = dram.tile([npc, MSG], f32, tag="zslice")
        zfull_dt = dram.tile([N_CORES * npc, MSG], f32, tag="zfull")

        # ---- phase 1: zdiff slice + allgather ----
        with tc.tile_pool(name="pz", bufs=2, space="PSUM") as pzp, \
                tc.tile_pool(name="zs", bufs=3) as zsp:
            for t in range(ntile):
                c0 = t * 128
                p = min(128, npc - c0)
                pz = pzp.tile([128, MSG], f32, tag="pz")
                for k in range(4):
                    nc.tensor.matmul(pz[:p, :], xh[k][:, c0:c0 + p], Wd[k][:],
                                     start=(k == 0), stop=False)
                nc.tensor.matmul(pz[:p, :], ones_t[:, c0:c0 + p], bdiff_t[:],
                                 start=False, stop=True)
                zs = zsp.tile([128, MSG], f32, tag="zs")
                nc.scalar.activation(zs[:p, :], pz[:p, :], Act.Identity)
                nc.sync.dma_start(zslice_dt[c0:c0 + p, :], zs[:p, :])

        if sim_single_core:
            # timing-equivalent stand-in for the AllGather in CoreSim
            for c in range(N_CORES):
                nc.sync.dma_start(zfull_dt[c * npc:(c + 1) * npc, :],
                                  zslice_dt[:])
        else:
            nc.gpsimd.collective_compute(
                "AllGather", Alu.bypass,
                replica_groups=[list(range(N_CORES))],
                ins=[zslice_dt.opt()], outs=[zfull_dt.opt()],
            )

        # ---- phase 2: edge bits + count aggregation ----
        nechunks = (nblk + CHB - 1) // CHB
        pcnt = None
        with tc.tile_pool(name="ed", bufs=3) as edp, \
                tc.tile_pool(name="pc", bufs=2, space="PSUM") as pcp:
            for ci in range(nechunks):
                b0 = ci * CHB
                b1 = min(nblk, b0 + CHB)
                w = b1 - b0
                A = edp.tile([128, w, MSG], f32, tag="A")
                B = edp.tile([128, w, MSG], f32, tag="B")
                # B = g1 - g0  (G0 input is pre-negated; CCE add during DMA)
                # G1 loads issue from the ACT sequencer to unload SP (both HWDGE)
                nc.scalar.dma_start(B[:], G1_d[:, b0:b1, :])
                nc.gpsimd.dma_start(B[:], G0_d[:, b0:b1, :], accum_op=Alu.add)
                # A = zdiff[src[slot]] -- in sub-gathers of <=1024 indices
                # (larger single dma_gather calls fault the exec unit)
                for j0 in range(0, w, 8):
                    j1 = min(w, j0 + 8)
                    nc.gpsimd.dma_gather(
                        out_ap=A[:, j0:j1, :], in_ap=zfull_dt[:],
                        idxs_ap=gidx_t[:, (b0 + j0) * 8:(b0 + j1) * 8],
                        num_idxs=(j1 - j0) * 128,
                        num_idxs_reg=(j1 - j0) * 128,
                        elem_size=MSG,
                    )
                bits = edp.tile([128, w, MSG], bf16, tag="bits")
                nc.vector.tensor_tensor(bits[:], A[:], B[:], op=Alu.is_ge)

                for b in range(b0, b1):
                    ck = int(blk_chunk[b])
                    wl = int(blk_woff[b]) - ck * NCHUNK
                    if b == chunk_first[ck]:
                        # sym-major count accumulator [64 syms, 128 nodes]
                        pcnt = pcp.tile([MSG, NCHUNK], f32, tag="pcnt")
                        nc.tensor.matmul(pcnt[:], zerot[:, :MSG], zerot[:],
                                         start=True, stop=False,
                                         skip_group_check=True)
                    nc.tensor.matmul(
                        pcnt[:, wl:wl + WIN], bits[:, b - b0, :], sel_t[:, b, :],
                        start=False, stop=(b == chunk_last[ck]),
                        skip_group_check=True)
                    if b == chunk_last[ck]:
                        p = min(128, npc - ck * NCHUNK)
                        nc.scalar.activation(
                            cntT[:, ck * NCHUNK:ck * NCHUNK + p], pcnt[:, :p],
                            Act.Identity)

        if debug_taps:
            nc.sync.dma_start(dbgz_d[:], zfull_dt[:])
            nc.sync.dma_start(dbgc_d[:], cntT[:])

        # ---- phase 3: GRU ----
        ngrp = (npc + GRP - 1) // GRP
        with tc.tile_pool(name="pg", bufs=2, space="PSUM") as pgp, \
                tc.tile_pool(name="pn", bufs=2, space="PSUM") as pnp, \
                tc.tile_pool(name="gr", bufs=2) as grp_:
            for g in range(ngrp):
                c0 = g * GRP
                n = min(GRP, npc - c0)
                sl = slice(c0, c0 + n)

                pd = pgp.tile([128, GRP], f32, tag="pd")
                nc.tensor.matmul(pd[:, :n], Wdd_t[:], cntT[:, sl],
                                 start=True, stop=False)
                nc.tensor.matmul(pd[:, :n], wodd_t[:], deg_t[:, sl],
                                 start=False, stop=True)
                decT = grp_.tile([128, GRP], f32, tag="dec")
                nc.scalar.activation(decT[:, :n], pd[:, :n], Act.Identity,
                                     bias=bdec_t[:])

                gates = []
                for f in range(4):  # r0, r1, z0, z1
                    pg = pgp.tile([128, GRP], f32, tag="pg")
                    fs = slice(f * 128, (f + 1) * 128)
                    nc.tensor.matmul(pg[:, :n], Wih[0][:, fs], xh[0][:, sl],
                                     start=True, stop=False)
                    nc.tensor.matmul(pg[:, :n], Wih[1][:, fs], xh[1][:, sl],
                                     start=False, stop=False)
                    nc.tensor.matmul(pg[:, :n], Wih[2][:, fs], decT[:, :n],
                                     start=False, stop=False)
                    nc.tensor.matmul(pg[:, :n], Whh[0][:, fs], xh[2][:, sl],
                                     start=False, stop=False)
                    nc.tensor.matmul(pg[:, :n], Whh[1][:, fs], xh[3][:, sl],
                                     start=False, stop=True)
                    gt = grp_.tile([128, GRP], f32, tag=f"gate{f}")
                    nc.scalar.activation(gt[:, :n], pg[:, :n], Act.Sigmoid,
                                         bias=bsum_t[:, f:f + 1])
                    gates.append(gt)

                for f2 in range(2):  # n-gate + output, 128-row halves
                    fs = slice((4 + f2) * 128, (5 + f2) * 128)
                    pgin = pnp.tile([128, GRP], f32, tag="pgin")
                    nc.tensor.matmul(pgin[:, :n], Wih[0][:, fs], xh[0][:, sl],
                                     start=True, stop=False)
                    nc.tensor.matmul(pgin[:, :n], Wih[1][:, fs], xh[1][:, sl],
                                     start=False, stop=False)
                    nc.tensor.matmul(pgin[:, :n], Wih[2][:, fs], decT[:, :n],
                                     start=False, stop=True)
                    pghn = pnp.tile([128, GRP], f32, tag="pghn")
                    nc.tensor.matmul(pghn[:, :n], Whh[0][:, fs], xh[2][:, sl],
                                     start=True, stop=False)
                    nc.tensor.matmul(pghn[:, :n], Whh[1][:, fs], xh[3][:, sl],
                                     start=False, stop=True)

                    hn = grp_.tile([128, GRP], f32, tag="hn")
                    nc.scalar.activation(hn[:, :n], pghn[:, :n], Act.Identity,
                                         bias=bhh6_t[:, 4 + f2:5 + f2])
                    t1 = grp_.tile([128, GRP], f32, tag="t1")
                    nc.vector.tensor_mul(t1[:, :n], gates[f2][:, :n], hn[:, :n])
                    t2 = grp_.tile([128, GRP], f32, tag="t2")
                    nc.vector.tensor_add(t2[:, :n], t1[:, :n], pgin[:, :n])
                    ng = grp_.tile([128, GRP], f32, tag="ng")
                    nc.scalar.activation(ng[:, :n], t2[:, :n], Act.Tanh,
                                         bias=bih6_t[:, 4 + f2:5 + f2])
                    u = grp_.tile([128, GRP], f32, tag="u")
                    nc.vector.tensor_sub(u[:, :n], xh[2 + f2][:, sl], ng[:, :n])
                    v = grp_.tile([128, GRP], f32, tag="v")
                    nc.vector.tensor_mul(v[:, :n], gates[2 + f2][:, :n], u[:, :n])
                    o = grp_.tile([128, GRP], f32, tag="o")
                    nc.vector.tensor_add(o[:, :n], ng[:, :n], v[:, :n])
                    nc.scalar.dma_start(outT_d[f2 * 128:(f2 + 1) * 128, sl],
                                        o[:, :n])

    return nc


def run_on_hw(sched, per_core, trace=False, debug_taps=False):
    from concourse.bass_utils import run_bass_kernel_spmd
    import ml_dtypes

    nc = build_nc(sched, debug_taps=debug_taps)
    nc.compile()
    in_maps = []
    for d in per_core:
        m = {}
        for k, v in d.items():
            if k == "gidx":
                continue  # golden-model only
            if k == "sel":
                m[k] = np.ascontiguousarray(v.astype(ml_dtypes.bfloat16))
            elif k == "gidx16":
                m[k] = np.ascontiguousarray(v.astype(np.int16))
            else:
                m[k] = np.ascontiguousarray(v.astype(np.float32))
        in_maps.append(m)
    res = run_bass_kernel_spmd(nc, in_maps, list(range(N_CORES)), trace=trace)
    return res


def kernel(**inputs):
    sched, per_core, cores = host_prep(**inputs)
    res = run_on_hw(sched, per_core)
    return host_post(res.results, cores)


if __name__ == "__main__":
    pass
